# revision 1
# baseline (speedup 1.0000x reference)
"""Dynamic per-sample 3x3 conv (kernel-predictor JointModel) on 8 trn2 cores.

Data-parallel: 16 samples per core. Per core:
  origin = x*std+mean    (DVE tensor_scalar, accum_out -> channel sums)
  feat   = mean(origin)  (sums -> gather -> fold halves)
  kern   = feat @ W1 + b1  (tiny PE matmul vs rearranged W1)
  out    = conv3x3(origin, kern) + bias   (block-diag PE matmuls,
           16 concurrent 32x32 tile_position, 9 shift taps + bias tap)

K-side partition: p = 32*strip + 6*sl + 2*ch + h
M-side (PSUM):    m = 6*sl + 2*o + h   (within 32*j col group)
strip 0..3 = samples 4*strip..4*strip+3; h = 112-row image half.
Padded half images [114, 226] bf16 per partition; conv taps are AP
column offsets (dy*226+dx) into them.
"""
import sys

import numpy as np

sys.path.insert(0, "/opt/trn_rl_repo")

_NCORE = 8
_BS = 16  # samples per core

_cache = {}


def _build(debug=False):
    import concourse.bass as bass
    import concourse.bacc as bacc
    import concourse.tile as tile
    from concourse import mybir

    f32 = mybir.dt.float32
    bf16 = mybir.dt.bfloat16
    MULT = mybir.AluOpType.mult
    ADD = mybir.AluOpType.add

    STD = [0.229, 0.224, 0.225]
    MEAN = [0.485, 0.456, 0.406]
    NPIX = 224 * 224

    nc = bacc.Bacc("TRN2", target_bir_lowering=False, debug=False)
    x_d = nc.dram_tensor("x", [_BS, 3, 224, 224], f32, kind="ExternalInput").ap()
    w1_d = nc.dram_tensor("W1", [3, 84], f32, kind="ExternalInput").ap()
    b1_d = nc.dram_tensor("b1", [84], f32, kind="ExternalInput").ap()
    out_d = nc.dram_tensor("out", [_BS, 3, 224, 224], f32, kind="ExternalOutput").ap()
    if debug:
        dbg_img = nc.dram_tensor("dbg_img", [128, 114, 226], bf16, kind="ExternalOutput").ap()
        dbg_feat = nc.dram_tensor("dbg_feat", [4, 16], f32, kind="ExternalOutput").ap()
        dbg_lhsw = nc.dram_tensor("dbg_lhsw", [128, 10, 24], bf16, kind="ExternalOutput").ap()
        dbg_sum = nc.dram_tensor("dbg_sum", [128, 8], f32, kind="ExternalOutput").ap()

    # x viewed (strip, sl, ch, h, y, x) - matches K-side partition order
    x_v = x_d.rearrange("(i sl) c (h y) w -> i sl c h y w", i=4, h=2)
    # out viewed (strip, wave, j, sl, o, h, r, c) - matches M-side order
    out_v = out_d.rearrange(
        "(i sl) o (h g j r) w -> i g j sl o h r w", i=4, h=2, j=4, r=2
    )
    # W1 cols idx=(o*3+ch)*9+off viewed (c, o, ch, off)
    w1_v = w1_d[:, 0:81].rearrange("c (o ch off) -> c o ch off", o=3, ch=3, off=9)
    b1_v = b1_d[0:81].rearrange("(o ch off) -> o ch off", o=3, ch=3, off=9)

    with tile.TileContext(nc) as tc:
        with (
            tc.tile_pool(name="big", bufs=1) as big,
            tc.tile_pool(name="stage", bufs=3) as stg_pool,
            tc.tile_pool(name="ev", bufs=4) as ev_pool,
            tc.tile_pool(name="small", bufs=1) as small,
            tc.tile_pool(name="psum2", bufs=2, space=bass.MemorySpace.PSUM) as pp2,
            tc.tile_pool(name="psum1", bufs=1, space=bass.MemorySpace.PSUM) as pp1,
        ):
            img = big.tile([128, 114, 226], bf16)
            ones = small.tile([128, 2, 224], bf16)
            lhsw = small.tile([128, 10, 24], bf16)
            stdv = small.tile([128, 1], f32)
            meanv = small.tile([128, 1], f32)
            sumbuf = small.tile([128, 8], f32)
            total = small.tile([128, 1], f32)
            g1 = small.tile([1, 4, 4, 3, 2], f32)  # (i; sl, ch, h)
            fs = small.tile([1, 4, 4, 4], f32)  # (i; ch4, sl); ch=3 row is ones
            featT = small.tile([4, 16], f32)
            w1r = small.tile([4, 3, 3, 10], f32)  # (c; o, ch, off)
            krb4 = small.tile([4, 4, 2, 3, 10, 6], bf16)  # (sl; i, hv, ch, off, oh)

            kr_ps = pp1.tile([4, 360], f32, tag="kr")

            nc.vector.memset(img[:], 0.0)
            nc.vector.memset(ones[:], 1.0)
            nc.vector.memset(lhsw[:], 0.0)
            nc.vector.memset(w1r[:], 0.0)
            nc.vector.memset(krb4[:], 0.0)
            nc.vector.memset(fs[:], 1.0)
            row_sm = small.tile([1, 2, 24], f32)  # [0]=std, [1]=mean pattern
            for ch in range(3):
                for h in range(2):
                    c0 = 2 * ch + h
                    nc.vector.memset(row_sm[0:1, 0, c0 : c0 + 19 : 6], STD[ch])
                    nc.vector.memset(row_sm[0:1, 1, c0 : c0 + 19 : 6], MEAN[ch])
            for i in range(4):
                nc.gpsimd.dma_start(stdv[32 * i : 32 * i + 24], row_sm[0:1, 0])
                nc.gpsimd.dma_start(meanv[32 * i : 32 * i + 24], row_sm[0:1, 1])

            # W1' load: conv taps + bias tap (off slot 9, ch=0 rows)
            nc.gpsimd.dma_start(w1r[0:3, :, :, 0:9], w1_v)
            nc.gpsimd.dma_start(w1r[3:4, :, :, 0:9], b1_v.unsqueeze(0))
            for o in range(3):
                nc.gpsimd.dma_start(
                    w1r[0:3, o, 0:1, 9:10], w1_d[:, 81 + o : 82 + o].unsqueeze(1)
                )
                nc.gpsimd.dma_start(
                    w1r[3:4, o, 0:1, 9:10],
                    b1_d[81 + o : 82 + o].unsqueeze(0).unsqueeze(0),
                )

            # ---------------- per-strip preamble ----------------
            for i in range(4):
                p0 = 32 * i
                # 8 chunks x 14 rows: img rows 1+14k..14+14k <-> y 112h+14k..
                for k in range(8):
                    st = stg_pool.tile([128, 14, 224], f32, tag="stage")
                    nc.gpsimd.dma_start(
                        st[p0 : p0 + 24], x_v[i, :, :, :, 14 * k : 14 * k + 14, :]
                    )
                    nc.scalar.activation(
                        img[p0 : p0 + 24, 1 + 14 * k : 15 + 14 * k, 1:225],
                        st[p0 : p0 + 24],
                        mybir.ActivationFunctionType.Identity,
                        bias=meanv[p0 : p0 + 24],
                        scale=stdv[p0 : p0 + 24],
                        accum_out=sumbuf[p0 : p0 + 24, k : k + 1],
                    )
                # halo rows, reusing the other half's denormed rows:
                # h=0 row 113 (=y112) <- h=1 row 1; h=1 row 0 (=y111) <- h=0 row 112
                nc.gpsimd.dma_start(
                    img[p0 : p0 + 23 : 2, 113:114, :], img[p0 + 1 : p0 + 24 : 2, 1:2, :]
                )
                nc.gpsimd.dma_start(
                    img[p0 + 1 : p0 + 24 : 2, 0:1, :], img[p0 : p0 + 23 : 2, 112:113, :]
                )
                # feat: fold chunk sums + halves, scale
                nc.vector.tensor_reduce(
                    total[p0 : p0 + 24], sumbuf[p0 : p0 + 24], mybir.AxisListType.X, ADD
                )
                nc.gpsimd.dma_start(g1[0:1, i], total[p0 : p0 + 24])
                g1v = g1[:].rearrange("p i sl ch h -> p i h ch sl")
                nc.vector.tensor_add(fs[0:1, i, 0:3], g1v[0:1, i, 0], g1v[0:1, i, 1])
                nc.scalar.mul(fs[0:1, i, 0:3], fs[0:1, i, 0:3], 1.0 / NPIX)
                nc.gpsimd.dma_start(featT[0:4, 4 * i : 4 * i + 4], fs[0:1, i])
                # kern[sl, (o ch off)] = featT.T @ W1r
                nc.tensor.matmul(
                    kr_ps[0:4, 90 * i : 90 * i + 90],
                    featT[0:4, 4 * i : 4 * i + 4],
                    w1r[:].rearrange("c o ch off -> c (o ch off)"),
                    start=True,
                    stop=True,
                )
                for h in range(2):
                    nc.vector.tensor_copy(
                        krb4[0:4, i, h, :, :, h : h + 5 : 2],
                        kr_ps[0:4, 90 * i : 90 * i + 90].rearrange(
                            "p (o ch off) -> p ch off o", o=3, ch=3, off=10
                        ),
                    )
                # scatter into block-diag LHS tiles
                for sl in range(4):
                    for h in range(2):
                        q = p0 + 6 * sl + h
                        nc.gpsimd.dma_start(
                            lhsw[q : q + 5 : 2, :, 6 * sl : 6 * sl + 6],
                            krb4[sl : sl + 1, i, h],
                        )

            if debug:
                nc.gpsimd.dma_start(dbg_img[:], img[:])
                nc.gpsimd.dma_start(dbg_feat[:], featT[:])
                nc.gpsimd.dma_start(dbg_lhsw[:], lhsw[:])
                nc.gpsimd.dma_start(dbg_sum[:], sumbuf[:])

            # ---------------- conv waves ----------------
            for w in range(14):
                for i in range(4):
                    p0 = 32 * i
                    if i < 3:
                        ps = pp2.tile([128, 2, 224], f32, tag=f"ps{i}")
                    else:
                        ps = pp1.tile([128, 2, 224], f32, tag="ps3")
                    for j in range(4):
                        g = 4 * w + j
                        q0 = 32 * j
                        for off in range(10):
                            if off < 9:
                                dy, dx = off // 3, off % 3
                                rhs = img[
                                    p0 : p0 + 24,
                                    2 * g + dy : 2 * g + dy + 2,
                                    dx : dx + 224,
                                ]
                            else:
                                rhs = ones[p0 : p0 + 24]
                            nc.tensor.matmul(
                                ps[q0 : q0 + 24],
                                lhsw[p0 : p0 + 24, off],
                                rhs,
                                start=(off == 0),
                                stop=(off == 9),
                                tile_position=(p0, q0),
                                skip_group_check=True,
                            )
                    ev = ev_pool.tile([128, 2, 224], f32, tag="ev")
                    nc.vector.tensor_copy(ev[:], ps[:])
                    for j in range(4):
                        nc.gpsimd.dma_start(out_v[i, w, j], ev[32 * j : 32 * j + 24])

    nc.compile()
    return nc


def _get_nc(debug=False):
    key = ("nc", debug)
    if key not in _cache:
        _cache[key] = _build(debug)
    return _cache[key]


def kernel(x: np.ndarray, W1: np.ndarray, b1: np.ndarray) -> np.ndarray:
    from concourse.bass_utils import run_bass_kernel_spmd

    nc = _get_nc()
    x = np.ascontiguousarray(x, dtype=np.float32)
    in_maps = [
        {
            "x": x[c * _BS : (c + 1) * _BS],
            "W1": np.ascontiguousarray(W1, dtype=np.float32),
            "b1": np.ascontiguousarray(b1, dtype=np.float32),
        }
        for c in range(_NCORE)
    ]
    res = run_bass_kernel_spmd(nc, in_maps, list(range(_NCORE)))
    outs = [res.results[c]["out"] for c in range(_NCORE)]
    return np.concatenate(outs, axis=0).astype(np.float32)

